# revision 14
# baseline (speedup 1.0000x reference)
"""CVMultiheadAttention Trainium2 kernel v2 (8 NeuronCores, SPMD).

Sharding: core c = (batch b = c//2, query-half sh = c%2). Each core computes
full K/V for its batch (duplicated across the pair), Q + attention rows for
its 512 queries over all 16 heads, then fc + residual + complex layernorm.
No collectives.

v2 structure (vs v1):
- bf16 operands everywhere on the matmul path (PSUM accum fp32); fp32 for
  residual + layernorm.  Halves DMA and SBUF, 1c/row PE transposes.
- Complex matmuls use component-stacked operands: scores contract over a
  128-deep [re;im] stack (2 matmuls instead of 4), A@V packs [o_r|o_i] in
  the stationary free dim (2 instead of 4).  Projections emit stacked/packed
  layouts directly via host-prepared interleaved weights.
- K/V stay in SBUF (no DRAM staging round-trip).
- K/Q projection for head h+1 overlaps attention for head h (PE stays busy
  while DVE/ACT/Pool run the MagMinMaxNorm chain).
- Normalization: custom DVE magsq+row-min fused op, Pool row-max,
  1/mag == sqrt(recip(magsq)) with the sqrt folded into the ACT pass that
  also applies the per-row mn scale: ratio = Sqrt(invmagsq*mnsq) = mn/mag.
  scale = (1-ratio)*invden applied on DVE in bf16.
"""
import sys
sys.path.insert(0, '/opt/trn_rl_repo')

from contextlib import ExitStack

import numpy as np

import concourse.bacc as bacc
import concourse.tile as tile
import concourse.mybir as mybir
from concourse import bass_utils

import concourse.dve_ops as dve_ops
from concourse.dve_spec import Spec, Src0, Src1, C0, lower, _has_src1, minn
from concourse.dve_uop import DveOpSpec


def _register(name, spec):
    if name in dve_ops._SUB_OPCODE_FOR_NAME:
        return next(op for op in dve_ops.OPS if op.name == name)
    row = dve_ops._CUSTOM_DVE_ROW_BASE + len(dve_ops.OPS)
    assert row < 0x20
    shas = {}
    for ver in ("v3", "v4"):
        tmp = DveOpSpec(name=name, opcode=row,
                        uops=lower(spec, ver=ver), rd1_en=_has_src1(spec))
        shas[ver] = tmp.sha(ver)
    op = dve_ops.DveOp(name, spec, subdim=False, uops_sha=shas)
    dve_ops.OPS.append(op)
    dve_ops.CUSTOM_DVE_SPECS[op.name] = op.spec
    dve_ops._SUB_OPCODE_FOR_NAME[op.name] = row
    return op


# out = in0^2 + in1^2, accum_out = row-min of out
MAGSQ_MIN_OP = _register("MAGSQMIN_ANT", Spec(
    body=Src0 * Src0 + Src1 * Src1,
    accum=minn,
    accum_init=C0,
    reference=lambda in0, in1, s0, s1, imm2:
        in0.astype(np.float32) ** 2 + in1.astype(np.float32) ** 2,
))

F32 = mybir.dt.float32
BF16 = mybir.dt.float16  # fp16: same PE/DVE rate as bf16, 8x finer mantissa
AX = mybir.AxisListType
OP = mybir.AluOpType
ACTF = mybir.ActivationFunctionType

B, S, D, H, DK, DV = 4, 1024, 1024, 16, 64, 64
P = 128
SQ = S // 2          # queries per core
NQT = SQ // P        # 4 q-tiles per core
NKT = D // P         # 8 contraction tiles over D
EPS = 1e-6

_CACHE = {}


def _build(reps=1, phases=3):
    nc = bacc.Bacc("TRN2", target_bir_lowering=False, debug=False, num_devices=8)

    def din(name, shape, dt=BF16):
        return nc.dram_tensor(name, shape, dt, kind="ExternalInput").ap()

    # per-core transposed activations (bf16)
    xq_rT = din("xq_rT", [D, SQ]); xq_iT = din("xq_iT", [D, SQ])
    xk_rT = din("xk_rT", [D, S]);  xk_iT = din("xk_iT", [D, S])
    xv_rT = din("xv_rT", [D, S]);  xv_iT = din("xv_iT", [D, S])
    # interleaved weight stacks [D, H*128]: per head [w_rT | w_iT] / [-w_iT | w_rT]
    wqA = din("wqA", [D, H * P]); wqB = din("wqB", [D, H * P])
    wkA = din("wkA", [D, H * P]); wkB = din("wkB", [D, H * P])
    wvA = din("wvA", [D, H * P]); wvB = din("wvB", [D, H * P])
    # fc stacks [H*128, D]: per head [fc_rT rows ; -fc_iT rows] / [fc_iT ; fc_rT]
    fcA = din("fcA", [H * P, D]); fcB = din("fcB", [H * P, D])
    # residual rows + LN params (fp32)
    resid_r = din("resid_r", [SQ, D], F32); resid_i = din("resid_i", [SQ, D], F32)
    g_rr = din("g_rr", [1, D], F32); g_ri = din("g_ri", [1, D], F32)
    g_ii = din("g_ii", [1, D], F32)
    b_r = din("b_r", [1, D], F32); b_i = din("b_i", [1, D], F32)
    ident = din("ident", [P, P])  # bf16 identity for PE transpose

    out = nc.dram_tensor("out", [2, SQ, D], F32, kind="ExternalOutput").ap()

    with tile.TileContext(nc) as tc, ExitStack() as glob_ctx:
        glob = glob_ctx.enter_context(tc.tile_pool(name="glob", bufs=1))

        def body():
            idt = glob.tile([P, P], BF16, tag="idt")
            nc.sync.dma_start(idt[:], ident)
            # packed attention output per head: [ (o_r dv 64 | o_i dv 64), h, q ]
            o_packT = glob.tile([P, H, SQ], BF16, tag="opk")
            # packed V: [k-part, k-chunk, head, (v_r 64 | v_i 64)]
            Vp1 = glob.tile([P, NKT, H, P], BF16, tag="vp1")

            # ---------------- Phase A: V projection ----------------
            with ExitStack() as ctx:
                xpool = ctx.enter_context(tc.tile_pool(name="xv", bufs=1))
                wpool = ctx.enter_context(tc.tile_pool(name="wv", bufs=2))
                ppool = ctx.enter_context(
                    tc.tile_pool(name="vps", bufs=4, space="PSUM"))
                xvr = xpool.tile([P, NKT, S], BF16, tag="xvr")
                xvi = xpool.tile([P, NKT, S], BF16, tag="xvi")
                nc.sync.dma_start(xvr[:], xv_rT.rearrange("(t p) k -> p t k", p=P))
                nc.sync.dma_start(xvi[:], xv_iT.rearrange("(t p) k -> p t k", p=P))
                for hg in range(4):  # 4-head groups
                    sl = slice(hg * 512, (hg + 1) * 512)
                    wva = wpool.tile([P, NKT, 512], BF16, tag="wva")
                    wvb = wpool.tile([P, NKT, 512], BF16, tag="wvb")
                    nc.sync.dma_start(wva[:], wvA[:, sl].rearrange("(t p) j -> p t j", p=P))
                    nc.sync.dma_start(wvb[:], wvB[:, sl].rearrange("(t p) j -> p t j", p=P))
                    for mt in range(NKT):  # key chunks
                        pv = ppool.tile([P, 512], F32, tag="pv")
                        for kt in range(NKT):
                            nc.tensor.matmul(pv[:], xvr[:, kt, mt * P:(mt + 1) * P],
                                             wva[:, kt, :], start=kt == 0, stop=False)
                            nc.tensor.matmul(pv[:], xvi[:, kt, mt * P:(mt + 1) * P],
                                             wvb[:, kt, :],
                                             start=False, stop=kt == NKT - 1)
                        nc.scalar.copy(Vp1[:, mt, hg * 4:(hg + 1) * 4, :], pv[:])

            # ---------------- Phase B: per-head K/Q proj + attention -------
            if phases < 2:
                return
            with ExitStack() as ctx:
                xpool = ctx.enter_context(tc.tile_pool(name="xkq", bufs=1))
                wpool = ctx.enter_context(tc.tile_pool(name="wkq", bufs=2))
                kpool = ctx.enter_context(tc.tile_pool(name="kst", bufs=2))
                qpool = ctx.enter_context(tc.tile_pool(name="qst", bufs=2))
                vpool = ctx.enter_context(tc.tile_pool(name="vp2", bufs=2))
                spool = ctx.enter_context(tc.tile_pool(name="scw", bufs=2))
                npool = ctx.enter_context(tc.tile_pool(name="nrm", bufs=2))
                tpool = ctx.enter_context(tc.tile_pool(name="ntp", bufs=1))
                sml = ctx.enter_context(tc.tile_pool(name="sml", bufs=4))
                psc = ctx.enter_context(
                    tc.tile_pool(name="psc", bufs=2, space="PSUM"))
                ptr = ctx.enter_context(
                    tc.tile_pool(name="ptr", bufs=1, space="PSUM"))
                pkq = ctx.enter_context(
                    tc.tile_pool(name="pkq", bufs=1, space="PSUM"))
                pav = ctx.enter_context(
                    tc.tile_pool(name="pav", bufs=1, space="PSUM"))

                xkr = xpool.tile([P, NKT, S], BF16, tag="xkr")
                xki = xpool.tile([P, NKT, S], BF16, tag="xki")
                xqr = xpool.tile([P, NKT, SQ], BF16, tag="xqr")
                xqi = xpool.tile([P, NKT, SQ], BF16, tag="xqi")
                nc.sync.dma_start(xkr[:], xk_rT.rearrange("(t p) k -> p t k", p=P))
                nc.sync.dma_start(xki[:], xk_iT.rearrange("(t p) k -> p t k", p=P))
                nc.sync.dma_start(xqr[:], xq_rT.rearrange("(t p) k -> p t k", p=P))
                nc.sync.dma_start(xqi[:], xq_iT.rearrange("(t p) k -> p t k", p=P))

                wk_sl = {}
                wq_sl = {}
                for h in range(H):
                    hp = h % 2  # position within the 2-head weight slab
                    if hp == 0:  # load 2-head weight slabs
                        sl = slice(h * P, (h + 2) * P)
                        wka = wpool.tile([P, NKT, 2 * P], BF16, tag="wka")
                        wkb = wpool.tile([P, NKT, 2 * P], BF16, tag="wkb")
                        wqa = wpool.tile([P, NKT, 2 * P], BF16, tag="wqa")
                        wqb = wpool.tile([P, NKT, 2 * P], BF16, tag="wqb")
                        nc.sync.dma_start(wka[:], wkA[:, sl].rearrange("(t p) j -> p t j", p=P))
                        nc.sync.dma_start(wkb[:], wkB[:, sl].rearrange("(t p) j -> p t j", p=P))
                        nc.sync.dma_start(wqa[:], wqA[:, sl].rearrange("(t p) j -> p t j", p=P))
                        nc.sync.dma_start(wqb[:], wqB[:, sl].rearrange("(t p) j -> p t j", p=P))
                        wk_sl, wq_sl = (wka, wkb), (wqa, wqb)
                    wsl = slice(hp * P, (hp + 1) * P)

                    # K projection: Kst = [k_r(h) ; k_i(h)] stacked on partitions
                    Kst = kpool.tile([P, S], BF16, tag="kst")
                    for nt in range(2):
                        nsl = slice(nt * 512, (nt + 1) * 512)
                        pk = pkq.tile([P, 512], F32, tag="pk")
                        for kt in range(NKT):
                            nc.tensor.matmul(pk[:], wk_sl[0][:, kt, wsl],
                                             xkr[:, kt, nsl], start=kt == 0, stop=False)
                            nc.tensor.matmul(pk[:], wk_sl[1][:, kt, wsl],
                                             xki[:, kt, nsl],
                                             start=False, stop=kt == NKT - 1)
                        nc.scalar.copy(Kst[:, nsl], pk[:])

                    # Q projection: qs1 = [q_r ; q_i] (then lower half negated)
                    qs1 = qpool.tile([P, SQ], BF16, tag="qs1")
                    qs2 = qpool.tile([P, SQ], BF16, tag="qs2")
                    pq = pkq.tile([P, SQ], F32, tag="pq")
                    for kt in range(NKT):
                        nc.tensor.matmul(pq[:], wq_sl[0][:, kt, wsl],
                                         xqr[:, kt, :], start=kt == 0, stop=False)
                        nc.tensor.matmul(pq[:], wq_sl[1][:, kt, wsl],
                                         xqi[:, kt, :], start=False, stop=kt == NKT - 1)
                    nc.scalar.copy(qs1[:], pq[:])
                    # qs2 = [q_i ; q_r] via SBUF-SBUF DMA partition swap
                    nc.sync.dma_start(qs2[0:64, :], qs1[64:128, :])
                    nc.sync.dma_start(qs2[64:128, :], qs1[0:64, :])
                    # qs1 lower half -> -q_i (after qs2 copied it)
                    nc.gpsimd.tensor_scalar(qs1[64:128, :], qs1[64:128, :],
                                            -1.0, None, OP.mult)

                    # Vpack2(h) = [-v_i | v_r] built from Vp1 on Pool
                    Vp2 = vpool.tile([P, NKT, P], BF16, tag="vp2")
                    nc.gpsimd.tensor_scalar(Vp2[:, :, 0:64], Vp1[:, :, h, 64:128],
                                            -1.0, None, OP.mult)
                    nc.gpsimd.tensor_scalar(Vp2[:, :, 64:128], Vp1[:, :, h, 0:64],
                                            1.0, None, OP.mult)

                    nreT = tpool.tile([P, NKT, SQ], BF16, tag="nreT")
                    nimT = tpool.tile([P, NKT, SQ], BF16, tag="nimT")

                    for qt in range(NQT):
                        qsl = slice(qt * P, (qt + 1) * P)
                        # scores: 128-deep stacked contraction, one matmul per
                        # (comp, k-half)
                        ps_re = psc.tile([P, S], F32, tag="ps")
                        ps_im = psc.tile([P, S], F32, tag="ps")
                        for nt in range(2):
                            nsl = slice(nt * 512, (nt + 1) * 512)
                            nc.tensor.matmul(ps_re[:, nsl], qs1[:, qsl],
                                             Kst[:, nsl], start=True, stop=True)
                            nc.tensor.matmul(ps_im[:, nsl], qs2[:, qsl],
                                             Kst[:, nsl], start=True, stop=True)
                        sre = spool.tile([P, S], BF16, tag="sre")
                        sim = spool.tile([P, S], BF16, tag="sim")
                        nc.scalar.copy(sre[:], ps_re[:])
                        nc.scalar.copy(sim[:], ps_im[:])
                        # magsq + row min (DVE), row max (Pool)
                        magsq = npool.tile([P, S], F32, tag="ms")
                        mnsq = sml.tile([P, 1], F32, tag="mnsq")
                        mxsq = sml.tile([P, 1], F32, tag="mxsq")
                        nc.vector._custom_dve(MAGSQ_MIN_OP, out=magsq[:],
                                              in0=sre[:], in1=sim[:],
                                              s0=3.0e38, accum_out=mnsq[:])
                        nc.vector.tensor_reduce(mxsq[:], magsq[:], AX.X, OP.max)
                        invms = npool.tile([P, S], F32, tag="ims")
                        nc.vector.reciprocal_approx_fast(out=invms[:], in_=magsq[:])
                        mn = sml.tile([P, 1], F32, tag="mn")
                        mx = sml.tile([P, 1], F32, tag="mx")
                        nc.scalar.activation(mn[:], mnsq[:], ACTF.Sqrt)
                        nc.scalar.activation(mx[:], mxsq[:], ACTF.Sqrt)
                        dmm = sml.tile([P, 1], F32, tag="dmm")
                        nc.vector.tensor_sub(dmm[:], mx[:], mn[:])
                        invden = sml.tile([P, 1], F32, tag="ivd")
                        nc.vector.reciprocal(invden[:], dmm[:])
                        ivdn = sml.tile([P, 1], F32, tag="ivdn")
                        nc.vector.tensor_scalar(ivdn[:], invden[:], -1.0, None,
                                                OP.mult)
                        # ratio = sqrt(invms * mnsq) = mn/mag   (bf16)
                        ratio = spool.tile([P, S], BF16, tag="rat")
                        nc.scalar.activation(ratio[:], invms[:], ACTF.Sqrt,
                                             scale=mnsq[:])
                        # sfac = (1 - ratio) * invden = ratio*(-invden) + invden
                        sfac = spool.tile([P, S], BF16, tag="sfac")
                        nc.vector.tensor_scalar(sfac[:], ratio[:], ivdn[:],
                                                invden[:], OP.mult, OP.add)
                        nre = npool.tile([P, S], BF16, tag="nre")
                        nim = npool.tile([P, S], BF16, tag="nim")
                        nc.vector.tensor_mul(nre[:], sre[:], sfac[:])
                        nc.gpsimd.tensor_mul(nim[:], sim[:], sfac[:])
                        # PE transpose 128x128 blocks (bf16)
                        for ci, (tsrc, dstT) in enumerate(((nre, nreT), (nim, nimT))):
                            for g in range(2):
                                tp = ptr.tile([P, 512], BF16, tag="tr")
                                for k4 in range(4):
                                    kt = g * 4 + k4
                                    nc.tensor.transpose(
                                        tp[:, k4 * P:(k4 + 1) * P],
                                        tsrc[:, kt * P:(kt + 1) * P], idt[:])
                                nc.scalar.copy(dstT[:, g * 4:(g + 1) * 4, qsl],
                                               tp[:])
                    # A@V packed: out [ (o_r | o_i), q ]
                    pv = pav.tile([P, SQ], F32, tag="pav")
                    for kt in range(NKT):
                        nc.tensor.matmul(pv[:], Vp1[:, kt, h, :], nreT[:, kt, :],
                                         start=kt == 0, stop=False)
                        nc.tensor.matmul(pv[:], Vp2[:, kt, :], nimT[:, kt, :],
                                         start=False, stop=kt == NKT - 1)
                    nc.scalar.copy(o_packT[:, h, :], pv[:])

            # ---------------- Phase C: fc + residual + LN ----------------
            if phases < 3:
                return
            with ExitStack() as ctx:
                wpool = ctx.enter_context(tc.tile_pool(name="fcw", bufs=1))
                lnp = ctx.enter_context(tc.tile_pool(name="ln", bufs=1))
                sml = ctx.enter_context(tc.tile_pool(name="lnsml", bufs=4))
                pfc = ctx.enter_context(
                    tc.tile_pool(name="pfc", bufs=6, space="PSUM"))
                gB = {}
                for nm, dr in (("g_rr", g_rr), ("g_ri", g_ri), ("g_ii", g_ii),
                               ("b_r", b_r), ("b_i", b_i)):
                    full = wpool.tile([P, D], F32, tag=nm + "B")
                    nc.sync.dma_start(full[:], dr.partition_broadcast(P))
                    gB[nm] = full
                fca = wpool.tile([P, H, D], BF16, tag="fca")
                fcb = wpool.tile([P, H, D], BF16, tag="fcb")
                nc.sync.dma_start(fca[:], fcA.rearrange("(t p) j -> p t j", p=P))
                nc.sync.dma_start(fcb[:], fcB.rearrange("(t p) j -> p t j", p=P))

                for st in range(NQT):
                    ssl = slice(st * P, (st + 1) * P)
                    yr = lnp.tile([P, D], F32, tag="yr")
                    yi = lnp.tile([P, D], F32, tag="yi")
                    rr = lnp.tile([P, D], F32, tag="rr")
                    ri = lnp.tile([P, D], F32, tag="ri")
                    nc.sync.dma_start(rr[:], resid_r[ssl, :])
                    nc.sync.dma_start(ri[:], resid_i[ssl, :])
                    for yt, rt, w2 in ((yr, rr, fca), (yi, ri, fcb)):
                        for nt in range(2):
                            nsl = slice(nt * 512, (nt + 1) * 512)
                            pq = pfc.tile([P, 512], F32, tag="p")
                            for hc in range(H):
                                nc.tensor.matmul(pq[:], o_packT[:, hc, ssl],
                                                 w2[:, hc, nsl],
                                                 start=hc == 0, stop=hc == H - 1)
                            nc.vector.tensor_add(yt[:, nsl], pq[:], rt[:, nsl])
                    # complex layernorm over D (fp32; same as v1)
                    sum_r = sml.tile([P, 1], F32, tag="w0")
                    sum_i = sml.tile([P, 1], F32, tag="w1")
                    nc.vector.tensor_reduce(sum_r[:], yr[:], AX.X, OP.add)
                    nc.vector.tensor_reduce(sum_i[:], yi[:], AX.X, OP.add)
                    mr = sml.tile([P, 1], F32, tag="w2")
                    mi = sml.tile([P, 1], F32, tag="w3")
                    nc.vector.tensor_scalar(mr[:], sum_r[:], 1.0 / D, None, OP.mult)
                    nc.vector.tensor_scalar(mi[:], sum_i[:], 1.0 / D, None, OP.mult)
                    rc = lnp.tile([P, D], F32, tag="rc")
                    ic = lnp.tile([P, D], F32, tag="ic")
                    nc.vector.tensor_scalar(rc[:], yr[:], mr[:], None, OP.subtract)
                    nc.vector.tensor_scalar(ic[:], yi[:], mi[:], None, OP.subtract)
                    u1 = lnp.tile([P, D], F32, tag="u1")
                    u2 = lnp.tile([P, D], F32, tag="u2")
                    scr = u1
                    vrr_s = sml.tile([P, 1], F32, tag="w4")
                    vii_s = sml.tile([P, 1], F32, tag="w5")
                    vri_s = sml.tile([P, 1], F32, tag="w6")
                    nc.scalar.activation(scr[:], rc[:], ACTF.Square,
                                         accum_out=vrr_s[:])
                    nc.scalar.activation(scr[:], ic[:], ACTF.Square,
                                         accum_out=vii_s[:])
                    nc.vector.tensor_mul(scr[:], rc[:], ic[:])
                    nc.vector.tensor_reduce(vri_s[:], scr[:], AX.X, OP.add)
                    vrr = sml.tile([P, 1], F32, tag="w7")
                    vii = sml.tile([P, 1], F32, tag="w8")
                    vri = sml.tile([P, 1], F32, tag="w9")
                    nc.vector.tensor_scalar(vrr[:], vrr_s[:], 1.0 / D, EPS,
                                            OP.mult, OP.add)
                    nc.vector.tensor_scalar(vii[:], vii_s[:], 1.0 / D, EPS,
                                            OP.mult, OP.add)
                    nc.vector.tensor_scalar(vri[:], vri_s[:], 1.0 / D, None, OP.mult)
                    pp = sml.tile([P, 1], F32, tag="w10")
                    qq = sml.tile([P, 1], F32, tag="w11")
                    det = sml.tile([P, 1], F32, tag="w12")
                    nc.vector.tensor_mul(pp[:], vrr[:], vii[:])
                    nc.vector.tensor_mul(qq[:], vri[:], vri[:])
                    nc.vector.tensor_sub(det[:], pp[:], qq[:])
                    sdet = sml.tile([P, 1], F32, tag="w13")
                    nc.scalar.activation(sdet[:], det[:], ACTF.Sqrt)
                    tin = sml.tile([P, 1], F32, tag="w14")
                    nc.vector.tensor_add(tin[:], vrr[:], vii[:])
                    tin2 = sml.tile([P, 1], F32, tag="w15")
                    nc.vector.tensor_scalar(tin2[:], sdet[:], 2.0, None, OP.mult)
                    tin3 = sml.tile([P, 1], F32, tag="w16")
                    nc.vector.tensor_add(tin3[:], tin[:], tin2[:])
                    tval = sml.tile([P, 1], F32, tag="w17")
                    nc.scalar.activation(tval[:], tin3[:], ACTF.Sqrt)
                    stv = sml.tile([P, 1], F32, tag="w18")
                    nc.vector.tensor_mul(stv[:], sdet[:], tval[:])
                    inv = sml.tile([P, 1], F32, tag="w19")
                    nc.vector.reciprocal(inv[:], stv[:])
                    wrr = sml.tile([P, 1], F32, tag="w20")
                    wii = sml.tile([P, 1], F32, tag="w21")
                    wri = sml.tile([P, 1], F32, tag="w22")
                    tmp = sml.tile([P, 1], F32, tag="w23")
                    nc.vector.tensor_add(tmp[:], vii[:], sdet[:])
                    nc.vector.tensor_mul(wrr[:], tmp[:], inv[:])
                    tmp2 = sml.tile([P, 1], F32, tag="w24")
                    nc.vector.tensor_add(tmp2[:], vrr[:], sdet[:])
                    nc.vector.tensor_mul(wii[:], tmp2[:], inv[:])
                    tmp3 = sml.tile([P, 1], F32, tag="w25")
                    nc.vector.tensor_mul(tmp3[:], vri[:], inv[:])
                    nc.vector.tensor_scalar(wri[:], tmp3[:], -1.0, None, OP.mult)
                    orr = rr
                    oii = ri
                    nc.vector.tensor_scalar(u1[:], rc[:], wrr[:], None, OP.mult)
                    nc.vector.tensor_scalar(u2[:], ic[:], wri[:], None, OP.mult)
                    nc.vector.tensor_add(orr[:], u1[:], u2[:])
                    nc.vector.tensor_scalar(u1[:], rc[:], wri[:], None, OP.mult)
                    nc.vector.tensor_scalar(u2[:], ic[:], wii[:], None, OP.mult)
                    nc.vector.tensor_add(oii[:], u1[:], u2[:])
                    t5 = rc
                    t6 = ic
                    outr = yr
                    outi = yi
                    nc.vector.tensor_mul(t5[:], orr[:], gB["g_rr"][:])
                    nc.vector.tensor_mul(t6[:], oii[:], gB["g_ri"][:])
                    nc.vector.tensor_add(t5[:], t5[:], t6[:])
                    nc.vector.tensor_add(outr[:], t5[:], gB["b_r"][:])
                    nc.vector.tensor_mul(t5[:], orr[:], gB["g_ri"][:])
                    nc.vector.tensor_mul(t6[:], oii[:], gB["g_ii"][:])
                    nc.vector.tensor_add(t5[:], t5[:], t6[:])
                    nc.vector.tensor_add(outi[:], t5[:], gB["b_i"][:])
                    nc.sync.dma_start(out[0, ssl, :], outr[:])
                    nc.sync.dma_start(out[1, ssl, :], outi[:])

        if reps == 1:
            body()
        else:
            with tc.For_i(0, reps, 1):
                body()

    nc.compile()
    return nc


def _get_module(reps=1):
    if reps not in _CACHE:
        _CACHE[reps] = _build(reps)
    return _CACHE[reps]


def _bf16(a):
    return np.ascontiguousarray(np.asarray(a, np.float32).astype(np.float16))


def _interleave(wr, wi, negate_first=False):
    """[w_r | w_i] per head along columns of [D, H*128] (inputs [D, H*64])."""
    D_, HJ = wr.shape
    out = np.empty((D_, 2 * HJ), np.float32)
    for h in range(H):
        out[:, h * 128:h * 128 + 64] = wr[:, h * 64:(h + 1) * 64]
        out[:, h * 128 + 64:(h + 1) * 128] = wi[:, h * 64:(h + 1) * 64]
    if negate_first:
        for h in range(H):
            out[:, h * 128:h * 128 + 64] *= -1.0
    return out


def _interleave_rows(ar, ai):
    """[a_r rows ; a_i rows] per head along rows of [H*128, D] (inputs [H*64, D])."""
    HJ, D_ = ar.shape
    out = np.empty((2 * HJ, D_), np.float32)
    for h in range(H):
        out[h * 128:h * 128 + 64, :] = ar[h * 64:(h + 1) * 64, :]
        out[h * 128 + 64:(h + 1) * 128, :] = ai[h * 64:(h + 1) * 64, :]
    return out


def make_in_maps(q_r, q_i, k_r, k_i, v_r, v_i, wq_r, wq_i, wk_r, wk_i,
                 wv_r, wv_i, fc_r, fc_i, gamma_rr, gamma_ri, gamma_ii,
                 beta_r, beta_i):
    f = np.float32
    ws = 1.0 / np.sqrt(np.float32(DK))
    T = lambda a: np.ascontiguousarray(np.asarray(a, f).T)
    wqrT = T(wq_r) * ws; wqiT = T(wq_i) * ws
    wkrT = T(wk_r); wkiT = T(wk_i)
    wvrT = T(wv_r); wviT = T(wv_i)
    # stacks: A = [w_rT | w_iT], B = [-w_iT | w_rT] per head
    wqA = _bf16(_interleave(wqrT, wqiT))
    wqB = _bf16(_interleave(wqiT, wqrT, negate_first=True))
    wkA = _bf16(_interleave(wkrT, wkiT))
    wkB = _bf16(_interleave(wkiT, wkrT, negate_first=True))
    wvA = _bf16(_interleave(wvrT, wviT))
    wvB = _bf16(_interleave(wviT, wvrT, negate_first=True))
    fcrT = np.ascontiguousarray(np.asarray(fc_r, f).T)  # [H*DV, D]
    fciT = np.ascontiguousarray(np.asarray(fc_i, f).T)
    fcA = _bf16(_interleave_rows(fcrT, -fciT))
    fcB = _bf16(_interleave_rows(fciT, fcrT))
    ident = np.eye(P, dtype=f)
    row = lambda a: np.ascontiguousarray(np.asarray(a, f).reshape(1, D))
    in_maps = []
    for c in range(8):
        b, sh = c // 2, c % 2
        qsl = slice(sh * SQ, (sh + 1) * SQ)
        in_maps.append({
            "xq_rT": _bf16(T(q_r[b])[:, qsl]), "xq_iT": _bf16(T(q_i[b])[:, qsl]),
            "xk_rT": _bf16(T(k_r[b])), "xk_iT": _bf16(T(k_i[b])),
            "xv_rT": _bf16(T(v_r[b])), "xv_iT": _bf16(T(v_i[b])),
            "wqA": wqA, "wqB": wqB, "wkA": wkA, "wkB": wkB,
            "wvA": wvA, "wvB": wvB, "fcA": fcA, "fcB": fcB,
            "resid_r": np.ascontiguousarray(np.asarray(q_r[b], f)[qsl]),
            "resid_i": np.ascontiguousarray(np.asarray(q_i[b], f)[qsl]),
            "g_rr": row(gamma_rr), "g_ri": row(gamma_ri), "g_ii": row(gamma_ii),
            "b_r": row(beta_r), "b_i": row(beta_i),
            "ident": _bf16(ident),
        })
    return in_maps


def assemble(results):
    full = np.zeros((2, B, S, D), np.float32)
    for c in range(8):
        b, sh = c // 2, c % 2
        full[:, b, sh * SQ:(sh + 1) * SQ, :] = results[c]["out"]
    return full


def kernel(**inputs):
    nc = _get_module()
    in_maps = make_in_maps(**inputs)
    res = bass_utils.run_bass_kernel_spmd(nc, in_maps, core_ids=list(range(8)))
    return assemble(res.results)


# revision 15
# speedup vs baseline: 1.2707x; 1.2707x over previous
"""CVMultiheadAttention Trainium2 kernel (8 NeuronCores, SPMD).

Sharding: core c = (batch b = c//2, query-half sh = c%2). Each core computes
full K/V projections for its batch (duplicated across the pair), Q projection
and attention rows for its 512 queries over all 16 heads, then fc + residual
+ complex layernorm on its rows. No collectives.

All matmuls run in float32r (TF32-like, full PE rate, ~1.5e-4 rel err).
Normalization math in fp32 on DVE/ACT. The attention MagMinMaxNorm
scale = (mag-mn)/((mx-mn)*mag + 1e-12) is computed as
s = invden*(1 - mn/mag) with invden = 1/(mx-mn) (eps negligible for
randn-scale data).
"""
import sys
sys.path.insert(0, '/opt/trn_rl_repo')

from contextlib import ExitStack

import numpy as np

import concourse.bacc as bacc
import concourse.tile as tile
import concourse.mybir as mybir
from concourse import bass_utils

import concourse.dve_ops as dve_ops
from concourse.dve_spec import Spec, Src0, Src1, lower, _has_src1
from concourse.dve_uop import DveOpSpec


def _register_magsq():
    if "MAGSQ_ANT" in dve_ops._SUB_OPCODE_FOR_NAME:
        return next(op for op in dve_ops.OPS if op.name == "MAGSQ_ANT")
    spec = Spec(
        body=Src0 * Src0 + Src1 * Src1,
        reference=lambda in0, in1, s0, s1, imm2:
            in0.astype(np.float32) ** 2 + in1.astype(np.float32) ** 2,
    )
    row = dve_ops._CUSTOM_DVE_ROW_BASE + len(dve_ops.OPS)
    assert row < 0x20
    shas = {}
    for ver in ("v3", "v4"):
        tmp = DveOpSpec(name="MAGSQ_ANT", opcode=row,
                        uops=lower(spec, ver=ver), rd1_en=_has_src1(spec))
        shas[ver] = tmp.sha(ver)
    op = dve_ops.DveOp("MAGSQ_ANT", spec, subdim=False, uops_sha=shas)
    dve_ops.OPS.append(op)
    dve_ops.CUSTOM_DVE_SPECS[op.name] = op.spec
    dve_ops._SUB_OPCODE_FOR_NAME[op.name] = row
    return op


MAGSQ_OP = _register_magsq()

F32 = mybir.dt.float32
F32R = mybir.dt.float32r
AX = mybir.AxisListType
OP = mybir.AluOpType
ACTF = mybir.ActivationFunctionType

B, S, D, H, DK, DV = 4, 1024, 1024, 16, 64, 64
P = 128
SQ = S // 2          # queries per core
NQT = SQ // P        # 4 q-tiles per core
NKT = D // P         # 8 contraction tiles over D
NHT = (H * DK) // P  # 8 partition tiles over head dim
EPS = 1e-6

_CACHE = {}


def _build(reps=1, phases=3):
    nc = bacc.Bacc("TRN2", target_bir_lowering=False, debug=False, num_devices=8)

    def din(name, shape, dt=F32R):
        return nc.dram_tensor(name, shape, dt, kind="ExternalInput").ap()

    # per-core transposed activations
    xq_rT = din("xq_rT", [D, SQ]); xq_iT = din("xq_iT", [D, SQ])
    xk_rT = din("xk_rT", [D, S]);  xk_iT = din("xk_iT", [D, S])
    xv_rT = din("xv_rT", [D, S]);  xv_iT = din("xv_iT", [D, S])
    # weights (transposed; wq* pre-scaled by 1/sqrt(DK))
    wqrT = din("wqrT", [D, H * DK]); wqiT = din("wqiT", [D, H * DK])
    wqiTn = din("wqiTn", [D, H * DK])
    wkrT = din("wkrT", [D, H * DK]); wkiT = din("wkiT", [D, H * DK])
    wkiTn = din("wkiTn", [D, H * DK])
    wvrT = din("wvrT", [D, H * DV]); wviT = din("wviT", [D, H * DV])
    wviTn = din("wviTn", [D, H * DV])
    fcrT = din("fcrT", [H * DV, D]); fciT = din("fciT", [H * DV, D])
    # residual rows + LN params
    resid_r = din("resid_r", [SQ, D], F32); resid_i = din("resid_i", [SQ, D], F32)
    g_rr = din("g_rr", [1, D], F32); g_ri = din("g_ri", [1, D], F32)
    g_ii = din("g_ii", [1, D], F32)
    b_r = din("b_r", [1, D], F32); b_i = din("b_i", [1, D], F32)
    ident = din("ident", [P, P])

    out = nc.dram_tensor("out", [2, SQ, D], F32, kind="ExternalOutput").ap()

    # DRAM staging for K/V (f32r)
    KrT_d = nc.dram_tensor("KrT_d", [H * DK, S], F32R, kind="Internal").ap()
    KiT_d = nc.dram_tensor("KiT_d", [H * DK, S], F32R, kind="Internal").ap()
    Vr_d = nc.dram_tensor("Vr_d", [S, H * DV], F32R, kind="Internal").ap()
    Vi_d = nc.dram_tensor("Vi_d", [S, H * DV], F32R, kind="Internal").ap()

    with tile.TileContext(nc) as tc, ExitStack() as glob_ctx:
        glob = glob_ctx.enter_context(tc.tile_pool(name="glob", bufs=1))

        def body():
            idt = glob.tile([P, P], F32R, tag="idt")
            nc.sync.dma_start(idt[:], ident)
            o_rT_sb = glob.tile([P, NHT, SQ], F32R, tag="orT")
            o_iT_sb = glob.tile([P, NHT, SQ], F32R, tag="oiT")

            # ---------------- Phase 1a: V projection ----------------
            # Vr[k,j] = sum_d xv_r[k,d] wvr[j,d] - xv_i[k,d] wvi[j,d]
            with ExitStack() as ctx:
                xpool = ctx.enter_context(tc.tile_pool(name="xv", bufs=1))
                wpool = ctx.enter_context(tc.tile_pool(name="wv", bufs=1))
                spool = ctx.enter_context(tc.tile_pool(name="vst", bufs=4))
                ppool = ctx.enter_context(
                    tc.tile_pool(name="vps", bufs=4, space="PSUM"))
                xvr = xpool.tile([P, NKT, S], F32R, tag="xvr")
                xvi = xpool.tile([P, NKT, S], F32R, tag="xvi")
                nc.sync.dma_start(xvr[:], xv_rT.rearrange("(t p) k -> p t k", p=P))
                nc.sync.dma_start(xvi[:], xv_iT.rearrange("(t p) k -> p t k", p=P))
                for nt in range(2):
                    wvr_s = wpool.tile([P, NKT, 512], F32R, tag="wvr")
                    wvi_s = wpool.tile([P, NKT, 512], F32R, tag="wvi")
                    wvin_s = wpool.tile([P, NKT, 512], F32R, tag="wvin")
                    sl = slice(nt * 512, (nt + 1) * 512)
                    nc.sync.dma_start(wvr_s[:], wvrT[:, sl].rearrange("(t p) j -> p t j", p=P))
                    nc.sync.dma_start(wvi_s[:], wviT[:, sl].rearrange("(t p) j -> p t j", p=P))
                    nc.sync.dma_start(wvin_s[:], wviTn[:, sl].rearrange("(t p) j -> p t j", p=P))
                    for mt in range(NKT):  # k-tiles of output
                        pr = ppool.tile([P, 512], F32, tag="p")
                        pi = ppool.tile([P, 512], F32, tag="p")
                        for kt in range(NKT):
                            st0 = kt == 0
                            nc.tensor.matmul(pr[:], xvr[:, kt, mt * P:(mt + 1) * P],
                                             wvr_s[:, kt, :], start=st0, stop=False)
                            nc.tensor.matmul(pr[:], xvi[:, kt, mt * P:(mt + 1) * P],
                                             wvin_s[:, kt, :],
                                             start=False, stop=kt == NKT - 1)
                            nc.tensor.matmul(pi[:], xvr[:, kt, mt * P:(mt + 1) * P],
                                             wvi_s[:, kt, :], start=st0, stop=False)
                            nc.tensor.matmul(pi[:], xvi[:, kt, mt * P:(mt + 1) * P],
                                             wvr_s[:, kt, :],
                                             start=False, stop=kt == NKT - 1)
                        sr = spool.tile([P, 512], F32R, tag="s")
                        si = spool.tile([P, 512], F32R, tag="s")
                        nc.scalar.copy(sr[:], pr[:])
                        nc.scalar.copy(si[:], pi[:])
                        nc.sync.dma_start(Vr_d[mt * P:(mt + 1) * P, sl], sr[:])
                        nc.sync.dma_start(Vi_d[mt * P:(mt + 1) * P, sl], si[:])

            # ---------------- Phase 1b: K projection ----------------
            # KrT[j,k] = sum_d wkr[j,d] xk_r[k,d]... (transposed out)
            with ExitStack() as ctx:
                xpool = ctx.enter_context(tc.tile_pool(name="xk", bufs=1))
                wpool = ctx.enter_context(tc.tile_pool(name="wk", bufs=4))
                spool = ctx.enter_context(tc.tile_pool(name="kst", bufs=4))
                ppool = ctx.enter_context(
                    tc.tile_pool(name="kps", bufs=4, space="PSUM"))
                xkr = xpool.tile([P, NKT, S], F32R, tag="xkr")
                xki = xpool.tile([P, NKT, S], F32R, tag="xki")
                nc.sync.dma_start(xkr[:], xk_rT.rearrange("(t p) k -> p t k", p=P))
                nc.sync.dma_start(xki[:], xk_iT.rearrange("(t p) k -> p t k", p=P))
                for mt in range(NHT):
                    msl = slice(mt * P, (mt + 1) * P)
                    wr = wpool.tile([P, NKT, P], F32R, tag="wr")
                    wi = wpool.tile([P, NKT, P], F32R, tag="wi")
                    win = wpool.tile([P, NKT, P], F32R, tag="win")
                    nc.sync.dma_start(wr[:], wkrT[:, msl].rearrange("(t p) j -> p t j", p=P))
                    nc.sync.dma_start(wi[:], wkiT[:, msl].rearrange("(t p) j -> p t j", p=P))
                    nc.sync.dma_start(win[:], wkiTn[:, msl].rearrange("(t p) j -> p t j", p=P))
                    for nt in range(2):
                        nsl = slice(nt * 512, (nt + 1) * 512)
                        pr = ppool.tile([P, 512], F32, tag="p")
                        pi = ppool.tile([P, 512], F32, tag="p")
                        for kt in range(NKT):
                            st0 = kt == 0
                            lw = kt == NKT - 1
                            nc.tensor.matmul(pr[:], wr[:, kt, :], xkr[:, kt, nsl],
                                             start=st0, stop=False)
                            nc.tensor.matmul(pr[:], win[:, kt, :], xki[:, kt, nsl],
                                             start=False, stop=lw)
                            nc.tensor.matmul(pi[:], wi[:, kt, :], xkr[:, kt, nsl],
                                             start=st0, stop=False)
                            nc.tensor.matmul(pi[:], wr[:, kt, :], xki[:, kt, nsl],
                                             start=False, stop=lw)
                        sr = spool.tile([P, 512], F32R, tag="s")
                        si = spool.tile([P, 512], F32R, tag="s")
                        nc.scalar.copy(sr[:], pr[:])
                        nc.scalar.copy(si[:], pi[:])
                        nc.sync.dma_start(KrT_d[msl, nsl], sr[:])
                        nc.sync.dma_start(KiT_d[msl, nsl], si[:])

            # ---------------- Phase 1c: Q projection (stays in SBUF) -------
            qctx = ExitStack()
            qpool = qctx.enter_context(tc.tile_pool(name="qpool", bufs=1))
            QrT_sb = qpool.tile([P, NHT, SQ], F32R, tag="QrT")
            QiT_sb = qpool.tile([P, NHT, SQ], F32R, tag="QiT")
            with ExitStack() as ctx:
                xpool = ctx.enter_context(tc.tile_pool(name="xq", bufs=1))
                wpool = ctx.enter_context(tc.tile_pool(name="wq", bufs=4))
                ppool = ctx.enter_context(
                    tc.tile_pool(name="qps", bufs=4, space="PSUM"))
                xqr = xpool.tile([P, NKT, SQ], F32R, tag="xqr")
                xqi = xpool.tile([P, NKT, SQ], F32R, tag="xqi")
                nc.sync.dma_start(xqr[:], xq_rT.rearrange("(t p) k -> p t k", p=P))
                nc.sync.dma_start(xqi[:], xq_iT.rearrange("(t p) k -> p t k", p=P))
                for mt in range(NHT):
                    msl = slice(mt * P, (mt + 1) * P)
                    wr = wpool.tile([P, NKT, P], F32R, tag="wr")
                    wi = wpool.tile([P, NKT, P], F32R, tag="wi")
                    win = wpool.tile([P, NKT, P], F32R, tag="win")
                    nc.sync.dma_start(wr[:], wqrT[:, msl].rearrange("(t p) j -> p t j", p=P))
                    nc.sync.dma_start(wi[:], wqiT[:, msl].rearrange("(t p) j -> p t j", p=P))
                    nc.sync.dma_start(win[:], wqiTn[:, msl].rearrange("(t p) j -> p t j", p=P))
                    pr = ppool.tile([P, SQ], F32, tag="p")
                    pi = ppool.tile([P, SQ], F32, tag="p")
                    for kt in range(NKT):
                        st0 = kt == 0
                        lw = kt == NKT - 1
                        nc.tensor.matmul(pr[:], wr[:, kt, :], xqr[:, kt, :],
                                         start=st0, stop=False)
                        nc.tensor.matmul(pr[:], win[:, kt, :], xqi[:, kt, :],
                                         start=False, stop=lw)
                        nc.tensor.matmul(pi[:], wi[:, kt, :], xqr[:, kt, :],
                                         start=st0, stop=False)
                        nc.tensor.matmul(pi[:], wr[:, kt, :], xqi[:, kt, :],
                                         start=False, stop=lw)
                    nc.scalar.copy(QrT_sb[:, mt, :], pr[:])
                    nc.scalar.copy(QiT_sb[:, mt, :], pi[:])

            # ---------------- Phase 2: attention ----------------
            if phases < 2:
                qctx.close()
                return
            with ExitStack() as ctx:
                kvp = ctx.enter_context(tc.tile_pool(name="kv", bufs=1))
                nrm = ctx.enter_context(tc.tile_pool(name="nrm", bufs=3))
                nrm2 = ctx.enter_context(tc.tile_pool(name="nrm2", bufs=2))
                sml = ctx.enter_context(tc.tile_pool(name="sml", bufs=4))
                ntp = ctx.enter_context(tc.tile_pool(name="ntp", bufs=1))
                psc = ctx.enter_context(
                    tc.tile_pool(name="psc", bufs=4, space="PSUM"))
                ptr = ctx.enter_context(
                    tc.tile_pool(name="ptr", bufs=2, space="PSUM"))
                pav = ctx.enter_context(
                    tc.tile_pool(name="pav", bufs=2, space="PSUM"))
                for h in range(H):
                    hp = (h % 2) * 64
                    ht = h // 2
                    hsl = slice(h * DK, (h + 1) * DK)
                    krt_t = kvp.tile([P, S], F32R, tag="krt")
                    kit_t = kvp.tile([P, S], F32R, tag="kit")
                    kitn_t = kvp.tile([P, S], F32R, tag="kitn")
                    krt = krt_t[hp:hp + 64, :]
                    kit = kit_t[hp:hp + 64, :]
                    kitn = kitn_t[hp:hp + 64, :]
                    nc.sync.dma_start(krt, KrT_d[hsl, :])
                    nc.sync.dma_start(kit, KiT_d[hsl, :])
                    nc.gpsimd.tensor_scalar(kitn, kit, -1.0, None, OP.mult)
                    vr = kvp.tile([P, NKT, DV], F32R, tag="vr")
                    vi = kvp.tile([P, NKT, DV], F32R, tag="vi")
                    vin = kvp.tile([P, NKT, DV], F32R, tag="vin")
                    nc.sync.dma_start(vr[:], Vr_d[:, hsl].rearrange("(t p) j -> p t j", p=P))
                    nc.sync.dma_start(vi[:], Vi_d[:, hsl].rearrange("(t p) j -> p t j", p=P))
                    nc.gpsimd.tensor_scalar(vin[:], vi[:], -1.0, None, OP.mult)

                    nreT = ntp.tile([P, NKT, SQ], F32R, tag="nreT")
                    nimT = ntp.tile([P, NKT, SQ], F32R, tag="nimT")

                    for qt in range(NQT):
                        qsl = slice(qt * P, (qt + 1) * P)
                        lq_r = QrT_sb[hp:hp + 64, ht, qsl]
                        lq_i = QiT_sb[hp:hp + 64, ht, qsl]
                        # scores in single-bank PSUM tiles, evicted to SBUF
                        # per half so PSUM frees at the finest grain
                        sre_sb = nrm.tile([P, S], F32, tag="sre_sb")
                        sim_sb = nrm.tile([P, S], F32, tag="sim_sb")
                        for nt in range(2):
                            nsl = slice(nt * 512, (nt + 1) * 512)
                            sre = psc.tile([P, 512], F32, tag="sc")
                            sim = psc.tile([P, 512], F32, tag="sc")
                            nc.tensor.matmul(sre[:], lq_r, krt[:, nsl],
                                             start=True, stop=False)
                            nc.tensor.matmul(sre[:], lq_i, kitn[:, nsl],
                                             start=False, stop=True)
                            nc.tensor.matmul(sim[:], lq_r, kit[:, nsl],
                                             start=True, stop=False)
                            nc.tensor.matmul(sim[:], lq_i, krt[:, nsl],
                                             start=False, stop=True)
                            nc.scalar.copy(sre_sb[:, nsl], sre[:])
                            nc.scalar.copy(sim_sb[:, nsl], sim[:])
                        magsq = nrm.tile([P, S], F32, tag="ms")
                        nc.vector._custom_dve(MAGSQ_OP, out=magsq[:],
                                              in0=sre_sb[:], in1=sim_sb[:])
                        mnsq = sml.tile([P, 1], F32, tag="mnsq")
                        mxsq = sml.tile([P, 1], F32, tag="mxsq")
                        nc.vector.tensor_reduce(mnsq[:], magsq[:], AX.X, OP.min)
                        nc.vector.tensor_reduce(mxsq[:], magsq[:], AX.X, OP.max)
                        mag = magsq
                        nc.scalar.activation(mag[:], magsq[:], ACTF.Sqrt)
                        mn = sml.tile([P, 1], F32, tag="mn")
                        mx = sml.tile([P, 1], F32, tag="mx")
                        nc.scalar.activation(mn[:], mnsq[:], ACTF.Sqrt)
                        nc.scalar.activation(mx[:], mxsq[:], ACTF.Sqrt)
                        dmm = sml.tile([P, 1], F32, tag="dmm")
                        nc.vector.tensor_sub(dmm[:], mx[:], mn[:])
                        invden = sml.tile([P, 1], F32, tag="invden")
                        nc.vector.reciprocal(invden[:], dmm[:])
                        t1 = sml.tile([P, 1], F32, tag="t1")
                        nc.vector.tensor_mul(t1[:], mn[:], invden[:])
                        t1n = sml.tile([P, 1], F32, tag="t1n")
                        nc.vector.tensor_scalar(t1n[:], t1[:], -1.0, None, OP.mult)
                        invmag = magsq
                        nc.vector.reciprocal_approx_fast(out=invmag[:], in_=mag[:])
                        sfac = magsq
                        nc.vector.tensor_scalar(sfac[:], invmag[:], t1n[:],
                                                invden[:], OP.mult, OP.add)
                        nre = nrm2.tile([P, S], F32R, tag="nre")
                        nim = nrm2.tile([P, S], F32R, tag="nim")
                        nc.vector.tensor_mul(nre[:], sre_sb[:], sfac[:])
                        nc.vector.tensor_mul(nim[:], sim_sb[:], sfac[:])
                        # transpose 128x128 blocks via PE
                        for comp, src, dstT in ((0, nre, nreT), (1, nim, nimT)):
                            for g in range(2):
                                tp = ptr.tile([P, 512], F32R, tag="tr")
                                for k4 in range(4):
                                    kt = g * 4 + k4
                                    nc.tensor.transpose(
                                        tp[:, k4 * P:(k4 + 1) * P],
                                        src[:, kt * P:(kt + 1) * P], idt[:])
                                nc.scalar.copy(
                                    dstT[:, g * 4:(g + 1) * 4, qsl], tp[:])
                    # A@V (transposed out): [dv, q]
                    avr = pav.tile([64, SQ], F32, tag="av")
                    avi = pav.tile([64, SQ], F32, tag="av")
                    for kt in range(NKT):
                        st0 = kt == 0
                        lw = kt == NKT - 1
                        nc.tensor.matmul(avr[:], vr[:, kt, :], nreT[:, kt, :],
                                         start=st0, stop=False)
                        nc.tensor.matmul(avr[:], vin[:, kt, :], nimT[:, kt, :],
                                         start=False, stop=lw)
                        nc.tensor.matmul(avi[:], vi[:, kt, :], nreT[:, kt, :],
                                         start=st0, stop=False)
                        nc.tensor.matmul(avi[:], vr[:, kt, :], nimT[:, kt, :],
                                         start=False, stop=lw)
                    nc.scalar.copy(o_rT_sb[hp:hp + 64, ht, :], avr[:])
                    nc.scalar.copy(o_iT_sb[hp:hp + 64, ht, :], avi[:])

            qctx.close()

            # ---------------- Phase 3: fc + residual + LN ----------------
            if phases < 3 and phases not in (25, 26):
                return
            with ExitStack() as ctx:
                wpool = ctx.enter_context(tc.tile_pool(name="fcw", bufs=1))
                lnp = ctx.enter_context(tc.tile_pool(name="ln", bufs=1))
                sml = ctx.enter_context(tc.tile_pool(name="lnsml", bufs=4))
                pfc = ctx.enter_context(
                    tc.tile_pool(name="pfc", bufs=6, space="PSUM"))
                gB = {}
                if phases >= 3:
                    for nm, dr in (("g_rr", g_rr), ("g_ri", g_ri), ("g_ii", g_ii),
                                   ("b_r", b_r), ("b_i", b_i)):
                        full = wpool.tile([P, D], F32, tag=nm + "B")
                        nc.sync.dma_start(full[:], dr.partition_broadcast(P))
                        gB[nm] = full
                fcr_sb = wpool.tile([P, NHT, D], F32R, tag="fcr")
                fci_sb = wpool.tile([P, NHT, D], F32R, tag="fci")
                nc.sync.dma_start(fcr_sb[:], fcrT.rearrange("(t p) j -> p t j", p=P))
                nc.sync.dma_start(fci_sb[:], fciT.rearrange("(t p) j -> p t j", p=P))
                o_iTn = wpool.tile([P, NHT, SQ], F32R, tag="oin")
                nc.vector.tensor_scalar(o_iTn[:], o_iT_sb[:], -1.0, None, OP.mult)

                for st in range(NQT):
                    ssl = slice(st * P, (st + 1) * P)
                    yr = lnp.tile([P, D], F32, tag="yr")
                    yi = lnp.tile([P, D], F32, tag="yi")
                    rr = lnp.tile([P, D], F32, tag="rr")
                    ri = lnp.tile([P, D], F32, tag="ri")
                    nc.sync.dma_start(rr[:], resid_r[ssl, :])
                    nc.sync.dma_start(ri[:], resid_i[ssl, :])
                    for comp, yt, rt, w2 in ((0, yr, rr, fcr_sb), (1, yi, ri, fci_sb)):
                        for nt in range(2):
                            nsl = slice(nt * 512, (nt + 1) * 512)
                            pq = pfc.tile([P, 512], F32, tag="p")
                            for kt in range(NHT):
                                st0 = kt == 0
                                lw = kt == NHT - 1
                                if comp == 0:
                                    nc.tensor.matmul(pq[:], o_rT_sb[:, kt, ssl],
                                                     fcr_sb[:, kt, nsl],
                                                     start=st0, stop=False)
                                    nc.tensor.matmul(pq[:], o_iTn[:, kt, ssl],
                                                     fci_sb[:, kt, nsl],
                                                     start=False, stop=lw)
                                else:
                                    nc.tensor.matmul(pq[:], o_rT_sb[:, kt, ssl],
                                                     fci_sb[:, kt, nsl],
                                                     start=st0, stop=False)
                                    nc.tensor.matmul(pq[:], o_iT_sb[:, kt, ssl],
                                                     fcr_sb[:, kt, nsl],
                                                     start=False, stop=lw)
                            nc.vector.tensor_add(yt[:, nsl], pq[:], rt[:, nsl])
                    if phases == 25:
                        nc.sync.dma_start(out[0, ssl, :], yr[:])
                        nc.sync.dma_start(out[1, ssl, :], yi[:])
                        continue
                    # complex layernorm over D
                    sum_r = sml.tile([P, 1], F32, tag="w0")
                    sum_i = sml.tile([P, 1], F32, tag="w1")
                    nc.vector.tensor_reduce(sum_r[:], yr[:], AX.X, OP.add)
                    nc.vector.tensor_reduce(sum_i[:], yi[:], AX.X, OP.add)
                    mr = sml.tile([P, 1], F32, tag="w2")
                    mi = sml.tile([P, 1], F32, tag="w3")
                    nc.vector.tensor_scalar(mr[:], sum_r[:], 1.0 / D, None, OP.mult)
                    nc.vector.tensor_scalar(mi[:], sum_i[:], 1.0 / D, None, OP.mult)
                    rc = lnp.tile([P, D], F32, tag="rc")
                    ic = lnp.tile([P, D], F32, tag="ic")
                    nc.vector.tensor_scalar(rc[:], yr[:], mr[:], None, OP.subtract)
                    nc.vector.tensor_scalar(ic[:], yi[:], mi[:], None, OP.subtract)
                    u1 = lnp.tile([P, D], F32, tag="u1")
                    u2 = lnp.tile([P, D], F32, tag="u2")
                    scr = u1
                    vrr_s = sml.tile([P, 1], F32, tag="w4")
                    vii_s = sml.tile([P, 1], F32, tag="w5")
                    vri_s = sml.tile([P, 1], F32, tag="w6")
                    nc.scalar.activation(scr[:], rc[:], ACTF.Square,
                                         accum_out=vrr_s[:])
                    nc.scalar.activation(scr[:], ic[:], ACTF.Square,
                                         accum_out=vii_s[:])
                    nc.vector.tensor_mul(scr[:], rc[:], ic[:])
                    nc.vector.tensor_reduce(vri_s[:], scr[:], AX.X, OP.add)
                    vrr = sml.tile([P, 1], F32, tag="w7")
                    vii = sml.tile([P, 1], F32, tag="w8")
                    vri = sml.tile([P, 1], F32, tag="w9")
                    nc.vector.tensor_scalar(vrr[:], vrr_s[:], 1.0 / D, EPS,
                                            OP.mult, OP.add)
                    nc.vector.tensor_scalar(vii[:], vii_s[:], 1.0 / D, EPS,
                                            OP.mult, OP.add)
                    nc.vector.tensor_scalar(vri[:], vri_s[:], 1.0 / D, None, OP.mult)
                    pp = sml.tile([P, 1], F32, tag="w10")
                    qq = sml.tile([P, 1], F32, tag="w11")
                    det = sml.tile([P, 1], F32, tag="w12")
                    nc.vector.tensor_mul(pp[:], vrr[:], vii[:])
                    nc.vector.tensor_mul(qq[:], vri[:], vri[:])
                    nc.vector.tensor_sub(det[:], pp[:], qq[:])
                    sdet = sml.tile([P, 1], F32, tag="w13")
                    nc.scalar.activation(sdet[:], det[:], ACTF.Sqrt)
                    tin = sml.tile([P, 1], F32, tag="w14")
                    nc.vector.tensor_add(tin[:], vrr[:], vii[:])
                    tin2 = sml.tile([P, 1], F32, tag="w15")
                    nc.vector.tensor_scalar(tin2[:], sdet[:], 2.0, None, OP.mult)
                    tin3 = sml.tile([P, 1], F32, tag="w16")
                    nc.vector.tensor_add(tin3[:], tin[:], tin2[:])
                    tval = sml.tile([P, 1], F32, tag="w17")
                    nc.scalar.activation(tval[:], tin3[:], ACTF.Sqrt)
                    stv = sml.tile([P, 1], F32, tag="w18")
                    nc.vector.tensor_mul(stv[:], sdet[:], tval[:])
                    inv = sml.tile([P, 1], F32, tag="w19")
                    nc.vector.reciprocal(inv[:], stv[:])
                    wrr = sml.tile([P, 1], F32, tag="w20")
                    wii = sml.tile([P, 1], F32, tag="w21")
                    wri = sml.tile([P, 1], F32, tag="w22")
                    tmp = sml.tile([P, 1], F32, tag="w23")
                    nc.vector.tensor_add(tmp[:], vii[:], sdet[:])
                    nc.vector.tensor_mul(wrr[:], tmp[:], inv[:])
                    tmp2 = sml.tile([P, 1], F32, tag="w24")
                    nc.vector.tensor_add(tmp2[:], vrr[:], sdet[:])
                    nc.vector.tensor_mul(wii[:], tmp2[:], inv[:])
                    tmp3 = sml.tile([P, 1], F32, tag="w25")
                    nc.vector.tensor_mul(tmp3[:], vri[:], inv[:])
                    nc.vector.tensor_scalar(wri[:], tmp3[:], -1.0, None, OP.mult)
                    # or_ = Wrr*rc + Wri*ic ; oi = Wri*rc + Wii*ic
                    orr = rr
                    oii = ri
                    nc.vector.tensor_scalar(u1[:], rc[:], wrr[:], None, OP.mult)
                    nc.vector.tensor_scalar(u2[:], ic[:], wri[:], None, OP.mult)
                    nc.vector.tensor_add(orr[:], u1[:], u2[:])
                    nc.vector.tensor_scalar(u1[:], rc[:], wri[:], None, OP.mult)
                    nc.vector.tensor_scalar(u2[:], ic[:], wii[:], None, OP.mult)
                    nc.vector.tensor_add(oii[:], u1[:], u2[:])
                    if phases == 26:
                        nc.sync.dma_start(out[0, ssl, :], orr[:])
                        nc.sync.dma_start(out[1, ssl, :], oii[:])
                        continue
                    # gamma/beta (general broadcast matrices)
                    t5 = rc
                    t6 = ic
                    outr = yr
                    outi = yi
                    nc.vector.tensor_mul(t5[:], orr[:], gB["g_rr"][:])
                    nc.vector.tensor_mul(t6[:], oii[:], gB["g_ri"][:])
                    nc.vector.tensor_add(t5[:], t5[:], t6[:])
                    nc.vector.tensor_add(outr[:], t5[:], gB["b_r"][:])
                    nc.vector.tensor_mul(t5[:], orr[:], gB["g_ri"][:])
                    nc.vector.tensor_mul(t6[:], oii[:], gB["g_ii"][:])
                    nc.vector.tensor_add(t5[:], t5[:], t6[:])
                    nc.vector.tensor_add(outi[:], t5[:], gB["b_i"][:])
                    nc.sync.dma_start(out[0, ssl, :], outr[:])
                    nc.sync.dma_start(out[1, ssl, :], outi[:])

        if reps == 1:
            body()
        else:
            with tc.For_i(0, reps, 1):
                body()

    nc.compile()
    return nc


def _get_module(reps=1):
    if reps not in _CACHE:
        _CACHE[reps] = _build(reps)
    return _CACHE[reps]


def make_in_maps(q_r, q_i, k_r, k_i, v_r, v_i, wq_r, wq_i, wk_r, wk_i,
                 wv_r, wv_i, fc_r, fc_i, gamma_rr, gamma_ri, gamma_ii,
                 beta_r, beta_i):
    f = np.float32
    ws = 1.0 / np.sqrt(np.float32(DK))
    T = lambda a: np.ascontiguousarray(np.asarray(a, f).T)
    wqrT = T(wq_r) * ws; wqiT = T(wq_i) * ws; wqiTn = -wqiT
    wkrT = T(wk_r); wkiT = T(wk_i); wkiTn = -wkiT
    wvrT = T(wv_r); wviT = T(wv_i); wviTn = -wviT
    fcrT = T(fc_r); fciT = T(fc_i)
    ident = np.eye(P, dtype=f)
    row = lambda a: np.ascontiguousarray(np.asarray(a, f).reshape(1, D))
    in_maps = []
    for c in range(8):
        b, sh = c // 2, c % 2
        qsl = slice(sh * SQ, (sh + 1) * SQ)
        in_maps.append({
            "xq_rT": T(q_r[b])[:, qsl].copy(), "xq_iT": T(q_i[b])[:, qsl].copy(),
            "xk_rT": T(k_r[b]), "xk_iT": T(k_i[b]),
            "xv_rT": T(v_r[b]), "xv_iT": T(v_i[b]),
            "wqrT": wqrT, "wqiT": wqiT, "wqiTn": wqiTn,
            "wkrT": wkrT, "wkiT": wkiT, "wkiTn": wkiTn,
            "wvrT": wvrT, "wviT": wviT, "wviTn": wviTn,
            "fcrT": fcrT, "fciT": fciT,
            "resid_r": np.ascontiguousarray(np.asarray(q_r[b], f)[qsl]),
            "resid_i": np.ascontiguousarray(np.asarray(q_i[b], f)[qsl]),
            "g_rr": row(gamma_rr), "g_ri": row(gamma_ri), "g_ii": row(gamma_ii),
            "b_r": row(beta_r), "b_i": row(beta_i),
            "ident": ident,
        })
    return in_maps


def assemble(results):
    full = np.zeros((2, B, S, D), np.float32)
    for c in range(8):
        b, sh = c // 2, c % 2
        full[:, b, sh * SQ:(sh + 1) * SQ, :] = results[c]["out"]
    return full


def kernel(**inputs):
    nc = _get_module()
    in_maps = make_in_maps(**inputs)
    res = bass_utils.run_bass_kernel_spmd(nc, in_maps, core_ids=list(range(8)))
    return assemble(res.results)

